# revision 43
# baseline (speedup 1.0000x reference)
"""Trainium2 Bass kernel for the topk-masking attention module.

Computation (per sample n):
    cams[k, hw] = relu(sum_c x[n, c, hw] * w[k, c])          # 1x1 conv, K=4
    thr[k]      = gama * max_hw(cams[k, :])
    dropped     = where(cams > thr, 0, cams)
    mean[hw]    = sum_k dropped[k, hw] / 4
    out[n,c,hw] = x[n,c,hw] * mean[hw]

Strategy (build_nc_v3, the shipped config): data-parallel over batch
N=32 across 8 NeuronCores (4 samples per core), with precision chosen
per data stream against the graded tolerance (rel_err < 2e-2):

- The host splits x = xh + xl/2048 with xh fp16 and xl the fp8-e4m3 of
  the scaled residual, and w into (wh fp16 | (w-wh)*2^11 fp16) plus
  w*2^5 in fp8.  HBM reads drop to 3 B/element (38.6 MB/core vs 51.4
  f32) while the conv stays effectively exact: the three single-pass
  matmul terms (wh*xh -> o_hi, wl*xh -> o_lo, w8*xl8 -> t3, each in its
  own base-0 PSUM tile, combined as o_hi + 2^-11 o_lo + 2^-16 t3 on
  ACT+DVE) reconstruct cams to ~7e-6 rms -- ZERO mask-threshold flips
  on the reference inputs, and 1-pass matmuls cost 1 PE cycle/row vs
  fp32's 4.
- The final multiply runs all-fp16 on DVE (2x mode), in place over the
  resident xh pair tiles, and the fp16 product is DMA'd out (25.7
  MB/core) and widened to f32 on the host after the gather.  Total HBM
  traffic 64.3 MB/core vs the f32 kernel's 102.8.
- DMA granularity: pair-sized loads/stores keep every HBM descriptor
  run at 12.5 KB/partition (8-piece loads measured 356 GB/s vs 401).
  Loads ride the sync ring, stores the scalar ring; two sample-NS-2
  pair stores are held back to bridge the final mask-chain stall.
- The mean matmul runs bf16 (masked/qlhs bf16), and the PSUM mean is
  staged to SBUF as fp16 by ACT so the multiply reads 16-bit operands.

Measured end to end: 281.1 us (f32 baseline) -> 196.5 us on 8-core
trn2, rel err 1.2e-3 (gate 2e-2), with DMA ~84% occupied at ~390 GB/s
per core and the PE ~70% busy underneath.
"""

import hashlib
import os
import sys

for _p in ("/opt/trn_rl_repo",):
    if _p not in sys.path:
        sys.path.insert(0, _p)

import numpy as np

N_CORES = 8
NFULL = 32            # full batch
NS = NFULL // N_CORES  # samples per core
C = 4096
K = 4
HW = 28 * 28          # 784
NCHUNK = C // 128     # 32
HALVES = ((0, 512), (512, HW))  # PSUM-bank-aligned column split

_CACHE = {}


def build_nc(n_pieces=8, x_bufs=16, out_bufs=8, cams_bufs=2, mean_bufs=2,
             store_engine="scalar", mean_to_sbuf=False, pe_filler=2,
             const_engine="scalar", hold_stores=2, spool_bufs=1,
             out_dt="float16", conv_fp32r=False, inplace_fp16=True,
             split_bf16=False, mean_dt="float32"):
    """Trace + schedule + compile the per-core Bass program.

    n_pieces: how many SBUF tiles one sample's x is split into (must
        divide 32); x_bufs slots of [128, 32/n_pieces, 784] each.
    out_dt: dtype of the DRAM output tensor.  float16 halves the store
        traffic (77.1 MB/core total vs 102.8 f32); the host widens the
        gathered result back to f32.  DVE writes the fp16 piece tiles
        (out_bufs slots) directly from the multiply.
    conv_fp32r: run the 1x1-conv matmuls with float32r operands
        (1 PE cycle/row vs fp32's 4 when the moving dim >= 256).
        Numerically a reduced-precision multiply -- flips some
        mask-threshold decisions; measure the final rel err on HW
        before trusting it.
    store_engine: which engine issues output DMAs ("sync"/"scalar"/
        "gpsimd") -- separate HWDGE ring from the loads avoids FIFO
        coupling.
    const_engine: ring for the w/gam/qlhs loads -- off the x-load ring
        so the first x piece transfer starts immediately.
    pe_filler: chunks of sample n+1 the PE runs ahead of sample n's mean
        matmul (in-order engine) to cover the relu/max/mask chain latency.
    hold_stores: defer this many stores of sample NS-2 and issue them at
        the start of sample NS-1's mask-chain window, bridging the DMA
        idle gap of that dependency stall.
    """
    from contextlib import ExitStack

    import concourse.bacc as bacc
    import concourse.tile as tile
    from concourse import mybir

    f32 = mybir.dt.float32
    bf16 = mybir.dt.bfloat16
    odt = getattr(mybir.dt, out_dt)
    # float32r is the same 4 bytes as float32; the BIR verifier requires the
    # whole producer chain (DRAM tensor -> DMA -> SBUF tile) to carry the
    # f32r tag for the matmult to consume it, so pick the dtype here.
    xdt = mybir.dt.float32r if conv_fp32r else f32
    nc = bacc.Bacc("TRN2", target_bir_lowering=False, debug=False,
                   num_devices=N_CORES)

    NP = n_pieces
    CPP = NCHUNK // NP  # chunks per piece

    if split_bf16:
        # x split on host into bf16 hi + lo planes (x == hi + lo to ~2^-17
        # relative): the conv runs as 3 single-pass bf16 matmuls per chunk
        # (wh*xh + wl*xh + wh*xl, dropping the ~2^-16-relative wl*xl term)
        # instead of fp32's effective 4 cycles/row -- same HBM read bytes,
        # 25% less PE time, and zero mask-threshold flips (measured: cams
        # rms error 4e-6 vs a flip band of ~1e-3).  The final multiply uses
        # only x_hi (out = fp16(x_hi * mean), ~2e-3 per-element) and runs
        # in place over the x_hi tile.
        xh_d = nc.dram_tensor("xh", [NS, C, HW], bf16, kind="ExternalInput")
        xl_d = nc.dram_tensor("xl", [NS, C, HW], bf16, kind="ExternalInput")
        w_d = nc.dram_tensor("w", [128, NCHUNK, 2, K], bf16,
                             kind="ExternalInput")
    else:
        x_d = nc.dram_tensor("x", [NS, C, HW], xdt, kind="ExternalInput")
        w_d = nc.dram_tensor("w", [128, NCHUNK, K], xdt, kind="ExternalInput")
    gam_d = nc.dram_tensor("gam", [K, 1], f32, kind="ExternalInput")
    qlhs_d = nc.dram_tensor("qlhs", [K, 128], f32, kind="ExternalInput")
    out_d = nc.dram_tensor("out", [NS, C, HW], odt, kind="ExternalOutput")

    # [NS, C, HW] viewed as [NS, 128(part), NCHUNK, HW]: partition p holds
    # the NCHUNK *adjacent* channels c = p*NCHUNK + j.  Each (partition,
    # piece) DMA run is then CPP*HW*dtype contiguous bytes and the w
    # host packing in make_in_maps is a plain reshape with the same mapping.
    if split_bf16:
        xh_v = xh_d.ap().rearrange("n (p j) hw -> n p j hw", p=128, j=NCHUNK)
        xl_v = xl_d.ap().rearrange("n (p j) hw -> n p j hw", p=128, j=NCHUNK)
    else:
        x_v = x_d.ap().rearrange("n (p j) hw -> n p j hw", p=128, j=NCHUNK)
    out_v = out_d.ap().rearrange("n (p j) hw -> n p j hw", p=128, j=NCHUNK)

    if isinstance(store_engine, (list, tuple)):
        store_engs = [getattr(nc, e) for e in store_engine]
    else:
        store_engs = [getattr(nc, store_engine)]
    const_eng = getattr(nc, const_engine)

    with tile.TileContext(nc) as tc, ExitStack() as ctx:
        consts = ctx.enter_context(tc.tile_pool(name="consts", bufs=1))
        # split mode allocates pair-sized load tiles, so halve the buf count
        # to keep the same two-samples-resident SBUF footprint.
        xb = x_bufs // 2 if split_bf16 else x_bufs
        xpool = ctx.enter_context(tc.tile_pool(name="xpool", bufs=xb))
        if split_bf16:
            lpool = ctx.enter_context(tc.tile_pool(name="lpool", bufs=xb))
        opool = ctx.enter_context(tc.tile_pool(name="opool", bufs=out_bufs))
        spool = ctx.enter_context(tc.tile_pool(name="spool", bufs=spool_bufs))
        tpool = ctx.enter_context(tc.tile_pool(name="tpool", bufs=2))
        cpsum = ctx.enter_context(
            tc.tile_pool(name="cpsum", bufs=cams_bufs, space="PSUM"))
        mpsum = ctx.enter_context(
            tc.tile_pool(name="mpsum", bufs=mean_bufs, space="PSUM"))

        xq_all = {}
        xl_all = {}
        ot_all = {}
        cams_all = {}

        def emit_loads(n):
            xq_all[n] = []
            xl_all[n] = []
            if split_bf16:
                # Load hi/lo in piece PAIRS: [128, 2*CPP, HW] bf16 tiles give
                # the same 12.5KB-per-partition contiguous DMA runs as the
                # f32 path (8 piece-sized loads measured 356 GB/s vs 401 for
                # pair-sized).  Multiply/store keep piece granularity via
                # half-tile views.
                for q2 in range(NP // 2):
                    sl = slice(q2 * 2 * CPP, (q2 + 1) * 2 * CPP)
                    t = xpool.tile([128, 2 * CPP, HW], bf16, tag="xh",
                                   name=f"xh_{n}_{q2}")
                    nc.sync.dma_start(t[:], xh_v[n][:, sl, :])
                    tl = lpool.tile([128, 2 * CPP, HW], bf16, tag="xl",
                                    name=f"xl_{n}_{q2}")
                    nc.sync.dma_start(tl[:], xl_v[n][:, sl, :])
                    for h in range(2):
                        hs = slice(h * CPP, (h + 1) * CPP)
                        xq_all[n].append(t[:, hs, :])
                        xl_all[n].append(tl[:, hs, :])
                return
            for q in range(NP):
                sl = slice(q * CPP, (q + 1) * CPP)
                t = xpool.tile([128, CPP, HW], xdt, tag="xq",
                               name=f"xq_{n}_{q}")
                nc.sync.dma_start(t[:], x_v[n][:, sl, :])
                xq_all[n].append(t)

        # x loads first in trace order; consts ride a separate ring.
        emit_loads(0)
        if split_bf16:
            w_sb = consts.tile([128, NCHUNK, 2, K], bf16, name="w_sb")
        else:
            w_sb = consts.tile([128, NCHUNK, K], xdt, name="w_sb")
        const_eng.dma_start(w_sb[:], w_d.ap())
        gam_sb = consts.tile([K, 1], f32, name="gam_sb")
        const_eng.dma_start(gam_sb[:], gam_d.ap())
        qlhs_sb = consts.tile([K, 128], f32, name="qlhs_sb")
        const_eng.dma_start(qlhs_sb[:], qlhs_d.ap())

        def as_f32(ap):
            # DVE consumes the f32r-tagged tiles as plain f32 bits.
            return ap.bitcast(f32) if conv_fp32r else ap

        def fp16_view(t):
            if split_bf16:
                # bf16 -> fp16 is the same element size: a plain in-place
                # dtype alias over the x_hi (half-tile view) AP.
                return t.bitcast(odt)
            # fp16 alias over the FIRST HALF of an f32 tile's bytes: the DVE
            # multiply streams in element order, so writing element i at byte
            # 2i while reading it at byte 4i never overtakes the reads.
            flat = t[:].rearrange("p j hw -> p (j hw)")
            return flat.bitcast(odt)[:, :CPP * HW].rearrange(
                "p (j hw) -> p j hw", j=CPP, hw=HW)

        def emit_chunk_mms(n, j_lo, j_hi):
            cams = cams_all[n]
            xq = xq_all[n]
            for j in range(j_lo, j_hi):
                q, jj = divmod(j, CPP)
                if split_bf16:
                    # wh*xh + wl*xh + wh*xl, accumulated in f32 PSUM.  The
                    # wh stationary is loaded twice back to back so the
                    # ldweights of each matmul hides under the previous
                    # matmul's rows (double-buffered PE weights).
                    terms = ((0, xq_all[n][q]), (1, xq_all[n][q]),
                             (0, xl_all[n][q]))
                    for t, (wi, src) in enumerate(terms):
                        for c0, c1 in HALVES:
                            nc.tensor.matmul(
                                cams[:, c0:c1],
                                w_sb[:, j, wi, :],
                                src[:, jj, c0:c1],
                                start=(j == 0 and t == 0),
                                stop=(j == NCHUNK - 1 and t == 2),
                            )
                else:
                    for c0, c1 in HALVES:
                        nc.tensor.matmul(
                            cams[:, c0:c1],
                            w_sb[:, j, :],
                            xq[q][:, jj, c0:c1],
                            start=(j == 0),
                            stop=(j == NCHUNK - 1),
                        )

        def emit_store(n, q):
            store_engs[q % len(store_engs)].dma_start(
                out_v[n][:, q * CPP:(q + 1) * CPP, :], ot_all[n][q])

        for n in range(NS):
            if n not in cams_all:
                cams_all[n] = cpsum.tile([K, HW], f32, tag="cams",
                                         name=f"cams_{n}")
            emit_chunk_mms(n, pe_filler if n > 0 else 0, NCHUNK)
            cams = cams_all[n]
            xq = xq_all[n]

            # Next sample's loads ahead of the mask chain in trace order so
            # the load ring's issue queue never drains behind it.
            if n + 1 < NS:
                emit_loads(n + 1)

            # relu on ACT (PSUM -> SBUF)
            r = spool.tile([K, HW], f32, tag="r", name=f"r_{n}")
            nc.scalar.activation(r[:], cams[:],
                                 mybir.ActivationFunctionType.Relu)
            # Held-back stores of sample n-1: dependency-free by now, they
            # keep the DMA engines fed through this sample's mask-chain
            # stall (the final sample especially).
            if n == NS - 1:
                for q in range(NP - hold_stores, NP):
                    emit_store(n - 1, q)
            # per-channel spatial max
            rmax = tpool.tile([K, 1], f32, tag="rmax", name=f"rmax_{n}")
            nc.vector.tensor_reduce(rmax[:], r[:], axis=mybir.AxisListType.X,
                                    op=mybir.AluOpType.max)
            # thr = gama * max
            thr = tpool.tile([K, 1], f32, tag="thr", name=f"thr_{n}")
            nc.vector.tensor_scalar(thr[:], rmax[:], gam_sb[:], None,
                                    op0=mybir.AluOpType.mult)
            # masked = (r <= thr) * r
            masked = spool.tile([K, HW], f32, tag="masked", name=f"masked_{n}")
            nc.vector.scalar_tensor_tensor(masked[:], r[:], thr[:], r[:],
                                           op0=mybir.AluOpType.is_le,
                                           op1=mybir.AluOpType.mult)
            # Keep PE busy while the DVE mask for sample n completes:
            # emit the first pe_filler chunk matmuls of sample n+1 ahead of
            # sample n's mean matmul in PE program order (in-order engine,
            # head-of-line blocking otherwise; also avoids a HAM idle gap).
            if n + 1 < NS and pe_filler:
                cams_all[n + 1] = cpsum.tile([K, HW], f32, tag="cams",
                                             name=f"cams_{n + 1}")
                emit_chunk_mms(n + 1, 0, pe_filler)

            # mean over k, broadcast to 128 partitions: qlhs (0.25) matmul
            meanb = mpsum.tile([128, HW], f32, tag="meanb", name=f"meanb_{n}")
            for c0, c1 in HALVES:
                nc.tensor.matmul(meanb[:, c0:c1], qlhs_sb[:],
                                 masked[:, c0:c1], start=True, stop=True)

            mean_src = meanb
            if mean_to_sbuf:
                # PSUM -> SBUF on ACT: GpSimd cannot read PSUM, and SBUF
                # operands are cheaper for DVE too.  mean_dt=float16 halves
                # the DVE read bytes of the broadcast operand (the mean is
                # re-rounded anyway by the fp16 store).
                mdt = getattr(mybir.dt, mean_dt)
                mean_sb = spool.tile([128, HW], mdt, tag="mean_sb",
                                     name=f"mean_sb_{n}")
                nc.scalar.activation(mean_sb[:], meanb[:],
                                     mybir.ActivationFunctionType.Copy)
                mean_src = mean_sb

            mb = mean_src.unsqueeze(1).broadcast_to([128, CPP, HW])
            ot_all[n] = []
            for q in range(NP):
                if inplace_fp16:
                    ot = fp16_view(xq[q])
                else:
                    ot = opool.tile([128, CPP, HW], odt, tag="ot",
                                    name=f"ot_{n}_{q}")[:]
                ot_all[n].append(ot)
                src = xq[q] if split_bf16 else as_f32(xq[q][:])
                nc.vector.tensor_tensor(ot, src, mb,
                                        op=mybir.AluOpType.mult)
                if n == NS - 2 and q >= NP - hold_stores:
                    continue  # deferred: see the hold_stores block above
                emit_store(n, q)

    nc.compile()
    nc._kernel_cfg = {"split_bf16": split_bf16}
    return nc


def build_nc_v3(cams_bufs=1, mean_bufs=1,
                store_engine="scalar", pe_filler=0, const_engine="scalar",
                hold_stores=2, spool_bufs=1, x_bufs=8, l_bufs=4,
                mean_dt="float16", mean_mm_bf16=False, t3_batch=False,
                pair_out=False, head_loads=True):
    """v3: fp16-hi + scaled-fp8-lo x planes, double-wide stationary.

    Host splits x = xh + xl/2048 (xh fp16, xl float8e4m3 of the scaled
    residual) and w into (wh fp16 | (w-wh)*2048 fp16) plus w*32 in fp8.
    Per chunk the conv is TWO single-pass matmuls instead of fp32's
    2x2-pass pair:
      rows 0:8  += [wh | wl*2^11]^T xh   (fp16, one [128,8] stationary --
                                          both w terms share the rows)
      rows 8:12 += (w*2^5)^T (xl*2^11)   (fp8)
    cams = o[0:4] + 2^-11 o[4:8] + 2^-16 o[8:12]  (two DVE ops)
    Measured on host against the reference inputs: zero mask-threshold
    flips, final rel err 3.6e-4.  Reads drop to 3 B/elem (38.6 MB/core),
    writes 25.7 MB fp16: 64.3 MB total HBM vs the f32 kernel's 102.8.
    The final multiply is all-fp16 (2x DVE mode), in place over the xh
    tiles, and the store DMAs run at piece granularity behind it.
    """
    from contextlib import ExitStack

    import concourse.bacc as bacc
    import concourse.tile as tile
    from concourse import mybir

    f32 = mybir.dt.float32
    f16 = mybir.dt.float16
    bf16 = mybir.dt.bfloat16
    f8 = mybir.dt.float8e4
    nc = bacc.Bacc("TRN2", target_bir_lowering=False, debug=False,
                   num_devices=N_CORES)

    NP = 8
    CPP = NCHUNK // NP  # 4 chunks per logical piece
    mmdt = bf16 if mean_mm_bf16 else f32

    xh_d = nc.dram_tensor("xh", [NS, C, HW], f16, kind="ExternalInput")
    xl_d = nc.dram_tensor("xl", [NS, C, HW], f8, kind="ExternalInput")
    w12_d = nc.dram_tensor("w12", [128, NCHUNK, 2 * K], f16,
                           kind="ExternalInput")
    w3_d = nc.dram_tensor("w3", [128, NCHUNK, K], f8, kind="ExternalInput")
    gam_d = nc.dram_tensor("gam", [K, 1], f32, kind="ExternalInput")
    qlhs_d = nc.dram_tensor("qlhs", [K, 128], mmdt, kind="ExternalInput")
    out_d = nc.dram_tensor("out", [NS, C, HW], f16, kind="ExternalOutput")

    xh_v = xh_d.ap().rearrange("n (p j) hw -> n p j hw", p=128, j=NCHUNK)
    xl_v = xl_d.ap().rearrange("n (p j) hw -> n p j hw", p=128, j=NCHUNK)
    out_v = out_d.ap().rearrange("n (p j) hw -> n p j hw", p=128, j=NCHUNK)

    if isinstance(store_engine, (list, tuple)):
        store_engs = [getattr(nc, e) for e in store_engine]
    else:
        store_engs = [getattr(nc, store_engine)]
    const_eng = getattr(nc, const_engine)

    with tile.TileContext(nc) as tc, ExitStack() as ctx:
        consts = ctx.enter_context(tc.tile_pool(name="consts", bufs=1))
        xpool = ctx.enter_context(tc.tile_pool(name="xpool", bufs=x_bufs))
        lpool = ctx.enter_context(tc.tile_pool(name="lpool", bufs=l_bufs))
        spool = ctx.enter_context(tc.tile_pool(name="spool", bufs=spool_bufs))
        tpool = ctx.enter_context(tc.tile_pool(name="tpool", bufs=2))
        cpsum = ctx.enter_context(
            tc.tile_pool(name="cpsum", bufs=cams_bufs, space="PSUM"))
        mpsum = ctx.enter_context(
            tc.tile_pool(name="mpsum", bufs=mean_bufs, space="PSUM"))

        xq_all = {}   # per sample: 8 half-views [128, CPP, HW] f16 of pairs
        xp_all = {}   # per sample: 4 full pair APs [128, 2*CPP, HW] f16
        xl_all = {}   # per sample: 8 quarter-views [128, CPP, HW] f8 of quads
        cams_all = {}
        t3_all = {}

        def emit_pair(n, q2, eng=None):
            sl = slice(q2 * 2 * CPP, (q2 + 1) * 2 * CPP)
            t = xpool.tile([128, 2 * CPP, HW], f16, tag="xh",
                           name=f"xh_{n}_{q2}")
            (eng or nc.sync).dma_start(t[:], xh_v[n][:, sl, :])
            xp_all[n].append(t[:])
            for h in range(2):
                xq_all[n].append(t[:, h * CPP:(h + 1) * CPP, :])

        def emit_quad(n, q4, eng=None):
            sl = slice(q4 * 4 * CPP, (q4 + 1) * 4 * CPP)
            tl = lpool.tile([128, 4 * CPP, HW], f8, tag="xl",
                            name=f"xl_{n}_{q4}")
            (eng or nc.sync).dma_start(tl[:], xl_v[n][:, sl, :])
            for h in range(4):
                xl_all[n].append(tl[:, h * CPP:(h + 1) * CPP, :])

        def emit_loads(n, head_eng=None):
            xq_all[n] = []
            xp_all[n] = []
            xl_all[n] = []
            # fp16 hi pairs + fp8 lo quads, all 12.5 KB/partition runs,
            # ordered by first PE use: the conv's fp8 term of piece 0 needs
            # quad 0 right after pair 0 (loading all pairs first measured
            # the first matmul gated at ~16us instead of ~5).  For sample 0
            # the first pair+quad can ride the const ring so the PE's first
            # wait clears as soon as they land, decoupled from the sync
            # queue's longer backlog.
            emit_pair(n, 0, head_eng)
            emit_quad(n, 0, head_eng)
            emit_pair(n, 1)
            emit_pair(n, 2)
            emit_quad(n, 1)
            emit_pair(n, 3)

        # consts first so they head the scalar queue, then sample 0's first
        # pair+quad on the same ring (see emit_loads).
        w12_sb = consts.tile([128, NCHUNK, 2 * K], f16, name="w12_sb")
        const_eng.dma_start(w12_sb[:], w12_d.ap())
        w3_sb = consts.tile([128, NCHUNK, K], f8, name="w3_sb")
        const_eng.dma_start(w3_sb[:], w3_d.ap())
        gam_sb = consts.tile([K, 1], f32, name="gam_sb")
        const_eng.dma_start(gam_sb[:], gam_d.ap())
        qlhs_sb = consts.tile([K, 128], mmdt, name="qlhs_sb")
        const_eng.dma_start(qlhs_sb[:], qlhs_d.ap())
        emit_loads(0, head_eng=(const_eng if head_loads else None))

        def emit_chunk_mms(n, j_lo, j_hi):
            o_hi, o_lo = cams_all[n]
            t3 = t3_all[n]
            for j in range(j_lo, j_hi):
                q, jj = divmod(j, CPP)
                for c0, c1 in HALVES:
                    # DVE PSUM reads need partition base 0 mod 32, so the
                    # wh and wl*2^11 terms go to separate base-0 tiles.
                    for out_t, w_lo_col in ((o_hi, 0), (o_lo, K)):
                        nc.tensor.matmul(
                            out_t[:, c0:c1],
                            w12_sb[:, j, w_lo_col:w_lo_col + K],
                            xq_all[n][q][:, jj, c0:c1],
                            start=(j == 0),
                            stop=(j == NCHUNK - 1),
                        )
                    if not t3_batch:
                        nc.tensor.matmul(
                            t3[:, c0:c1],
                            w3_sb[:, j, :],
                            xl_all[n][q][:, jj, c0:c1],
                            start=(j == 0),
                            stop=(j == NCHUNK - 1),
                        )
                if t3_batch and (j + 1) % CPP == 0:
                    # batch the fp8 term per piece to cut stationary-dtype
                    # flip-flopping on the PE
                    for j2 in range(j - CPP + 1, j + 1):
                        q2, jj2 = divmod(j2, CPP)
                        for c0, c1 in HALVES:
                            nc.tensor.matmul(
                                t3[:, c0:c1],
                                w3_sb[:, j2, :],
                                xl_all[n][q2][:, jj2, c0:c1],
                                start=(j2 == 0),
                                stop=(j2 == NCHUNK - 1),
                            )

        def emit_store(n, q):
            if pair_out:
                sl = slice(q * 2 * CPP, (q + 1) * 2 * CPP)
                store_engs[q % len(store_engs)].dma_start(
                    out_v[n][:, sl, :], xp_all[n][q])
                return
            store_engs[q % len(store_engs)].dma_start(
                out_v[n][:, q * CPP:(q + 1) * CPP, :], xq_all[n][q])

        NOUT = NP // 2 if pair_out else NP  # multiply/store granules

        def alloc_psum(n):
            cams_all[n] = (
                cpsum.tile([K, HW], f32, tag="ohi", name=f"ohi_{n}"),
                cpsum.tile([K, HW], f32, tag="olo", name=f"olo_{n}"),
            )
            t3_all[n] = cpsum.tile([K, HW], f32, tag="t3", name=f"t3_{n}")

        for n in range(NS):
            if n not in cams_all:
                alloc_psum(n)
            emit_chunk_mms(n, pe_filler if n > 0 else 0, NCHUNK)
            o_hi, o_lo = cams_all[n]

            if n + 1 < NS:
                emit_loads(n + 1)

            # cams = o_hi + 2^-11 o_lo + 2^-16 o_t3.  DVE may read only ONE
            # PSUM operand per op, so ACT stages the scaled o_lo to SBUF.
            tmp = spool.tile([K, HW], f32, tag="ctmp", name=f"ctmp_{n}")
            nc.scalar.activation(tmp[:], o_lo[:],
                                 mybir.ActivationFunctionType.Copy,
                                 scale=1.0 / 2048.0)
            tmp2 = spool.tile([K, HW], f32, tag="ctmp2", name=f"ctmp2_{n}")
            nc.vector.scalar_tensor_tensor(
                tmp2[:], o_hi[:], 1.0, tmp[:],
                op0=mybir.AluOpType.mult, op1=mybir.AluOpType.add)
            csb = spool.tile([K, HW], f32, tag="csb", name=f"csb_{n}")
            nc.vector.scalar_tensor_tensor(
                csb[:], t3_all[n][:], 1.0 / 65536.0, tmp2[:],
                op0=mybir.AluOpType.mult, op1=mybir.AluOpType.add)
            # relu on ACT (SBUF -> SBUF)
            r = spool.tile([K, HW], f32, tag="r", name=f"r_{n}")
            nc.scalar.activation(r[:], csb[:],
                                 mybir.ActivationFunctionType.Relu)
            if n == NS - 1:
                for q in range(NOUT - hold_stores, NOUT):
                    emit_store(n - 1, q)
            rmax = tpool.tile([K, 1], f32, tag="rmax", name=f"rmax_{n}")
            nc.vector.tensor_reduce(rmax[:], r[:], axis=mybir.AxisListType.X,
                                    op=mybir.AluOpType.max)
            thr = tpool.tile([K, 1], f32, tag="thr", name=f"thr_{n}")
            nc.vector.tensor_scalar(thr[:], rmax[:], gam_sb[:], None,
                                    op0=mybir.AluOpType.mult)
            masked = spool.tile([K, HW], mmdt, tag="masked",
                                name=f"masked_{n}")
            nc.vector.scalar_tensor_tensor(masked[:], r[:], thr[:], r[:],
                                           op0=mybir.AluOpType.is_le,
                                           op1=mybir.AluOpType.mult)
            if n + 1 < NS and pe_filler:
                alloc_psum(n + 1)
                emit_chunk_mms(n + 1, 0, pe_filler)

            meanb = mpsum.tile([128, HW], f32, tag="meanb", name=f"meanb_{n}")
            for c0, c1 in HALVES:
                nc.tensor.matmul(meanb[:, c0:c1], qlhs_sb[:],
                                 masked[:, c0:c1], start=True, stop=True)

            # PSUM -> SBUF fp16 mean so the multiply runs all-16-bit on DVE
            mdt = getattr(mybir.dt, mean_dt)
            mean_sb = spool.tile([128, HW], mdt, tag="mean_sb",
                                 name=f"mean_sb_{n}")
            nc.scalar.activation(mean_sb[:], meanb[:],
                                 mybir.ActivationFunctionType.Copy)

            gran = 2 * CPP if pair_out else CPP
            mb = mean_sb.unsqueeze(1).broadcast_to([128, gran, HW])
            srcs = xp_all[n] if pair_out else xq_all[n]
            for q in range(NOUT):
                # in place: fp16 xh view * fp16 mean -> same fp16 bytes
                nc.vector.tensor_tensor(srcs[q], srcs[q], mb,
                                        op=mybir.AluOpType.mult)
                if n == NS - 2 and q >= NOUT - hold_stores:
                    continue
                emit_store(n, q)

    nc.compile()
    nc._kernel_cfg = {"v3": True, "mean_mm_bf16": mean_mm_bf16}
    return nc


def _get_nc():
    if "nc" not in _CACHE:
        _CACHE["nc"] = build_nc_v3(t3_batch=True, hold_stores=2,
                                   mean_mm_bf16=True, pair_out=True)
    return _CACHE["nc"]


def make_in_maps(x, fc_weights, gama):
    """Shard/pack full numpy inputs into per-core input maps."""
    cfg = getattr(_get_nc(), "_kernel_cfg", {})
    x = np.ascontiguousarray(np.asarray(x, dtype=np.float32).reshape(NFULL, C, HW))
    fcw = np.asarray(fc_weights, dtype=np.float32).reshape(K, C)
    gam4 = np.full((K, 1), np.float32(np.asarray(gama)), dtype=np.float32)
    qlhs = np.full((K, 128), 0.25, dtype=np.float32)
    in_maps = []
    if cfg.get("v3"):
        import ml_dtypes

        e4 = ml_dtypes.float8_e4m3fn
        xh = x.astype(np.float16)
        xl8 = ((x - xh.astype(np.float32)) * 2048.0).astype(e4)
        wh = fcw.astype(np.float16)
        wls = ((fcw - wh.astype(np.float32)) * 2048.0).astype(np.float16)
        # w12[p, j, 0:4] = wh[k, c], w12[p, j, 4:8] = (w-wh)*2^11;
        # w3[p, j, k] = w[k, c]*2^5 in fp8; channel c = p*NCHUNK + j
        w12 = np.ascontiguousarray(np.concatenate(
            [wh.T.reshape(128, NCHUNK, K),
             wls.T.reshape(128, NCHUNK, K)], axis=2))
        w3 = np.ascontiguousarray(
            (fcw.T.reshape(128, NCHUNK, K) * 32.0).astype(e4))
        if cfg.get("mean_mm_bf16"):
            qlhs = qlhs.astype(ml_dtypes.bfloat16)
        for c in range(N_CORES):
            in_maps.append({
                "xh": xh[c * NS:(c + 1) * NS],
                "xl": xl8[c * NS:(c + 1) * NS],
                "w12": w12,
                "w3": w3,
                "gam": gam4,
                "qlhs": qlhs,
            })
        return in_maps
    if cfg.get("split_bf16"):
        import ml_dtypes

        bf16 = ml_dtypes.bfloat16
        xh = x.astype(bf16)
        xl = (x - xh.astype(np.float32)).astype(bf16)
        wh = fcw.astype(bf16)
        wl = (fcw - wh.astype(np.float32)).astype(bf16)
        # w2[p, j, i, k] = (wh if i == 0 else wl)[k, p*NCHUNK + j]
        w2 = np.ascontiguousarray(
            np.stack([wh.T.reshape(128, NCHUNK, K),
                      wl.T.reshape(128, NCHUNK, K)], axis=2))
        for c in range(N_CORES):
            in_maps.append({
                "xh": xh[c * NS:(c + 1) * NS],
                "xl": xl[c * NS:(c + 1) * NS],
                "w": w2,
                "gam": gam4,
                "qlhs": qlhs,
            })
        return in_maps
    # w_arr[p, j, k] = fcw[k, p*NCHUNK + j]  (channel c = p*NCHUNK + j,
    # matching the x view in build_nc)
    w_arr = np.ascontiguousarray(fcw.T.reshape(128, NCHUNK, K))
    for c in range(N_CORES):
        in_maps.append({
            "x": x[c * NS:(c + 1) * NS],
            "w": w_arr,
            "gam": gam4,
            "qlhs": qlhs,
        })
    return in_maps


def _strip_debug(obj):
    """Recursively blank debug-only fields (file paths / tracebacks) so the
    cache key is independent of where kernel.py lives on disk."""
    if isinstance(obj, dict):
        return {
            k: ("" if k in ("filename", "ant_traceback") else _strip_debug(v))
            for k, v in obj.items()
        }
    if isinstance(obj, list):
        return [_strip_debug(v) for v in obj]
    return obj


def _bass_module_cache_key(code, code_format):
    """Semantic cache key for a bass_exec HLO module, or None.

    Hashes the embedded BIR with debug-only fields blanked, plus the
    IO-name/arch config.  Any semantic difference changes the key; a
    path-only difference (same kernel traced from another directory)
    does not.
    """
    import base64
    import json

    if b"bass_exec" not in bytes(code) or bytes(code_format) != b"hlo":
        return None
    import libneuronxla.proto.hlo_pb2 as hlo_pb2
    from concourse import bass2jax

    proto = hlo_pb2.HloModuleProto.FromString(bytes(code))
    cfgs = [
        ins.backend_config
        for comp in proto.computations
        for ins in comp.instructions
        if ins.opcode == "custom-call" and ins.custom_call_target == "bass_exec"
    ]
    if len(cfgs) != 1:
        return None
    config = json.loads(base64.standard_b64decode(cfgs[0]))
    decomp = getattr(bass2jax, "_decompress_ant_bir", None)
    if decomp is None:
        return None
    bir = json.loads(decomp(config["ant_bir"]))
    h = hashlib.sha256()
    h.update(json.dumps(_strip_debug(bir), sort_keys=True).encode())
    h.update(json.dumps(
        [config.get("in_names"), config.get("out_names"),
         config.get("arch"), proto.name],
        sort_keys=True).encode())
    return h.hexdigest()


def _install_neff_cache():
    """Wrap concourse's neuronx_cc hook with a content-keyed NEFF cache.

    The stock hook recompiles the NEFF from scratch in every process
    (minutes for this kernel); the emitted BIR is deterministic modulo
    debug file paths, so a debug-stripped content hash makes repeat
    compiles of the identical module instant.
    """
    if _CACHE.get("cc_cached"):
        return
    try:
        from concourse import bass2jax

        inner = bass2jax.neuronx_cc_hook
        cache_dir = os.path.expanduser("~/.cache/bass_neff_cache")
        os.makedirs(cache_dir, exist_ok=True)

        def cached_hook(code, code_format, platform_version, file_prefix):
            path = None
            try:
                key = _bass_module_cache_key(code, code_format)
                if key is not None:
                    path = os.path.join(cache_dir, key)
                    if os.path.exists(path):
                        with open(path, "rb") as f:
                            return 0, f.read()
            except Exception:
                path = None
            ret, data = inner(code, code_format, platform_version, file_prefix)
            if path is not None and ret == 0:
                try:
                    tmp = f"{path}.tmp{os.getpid()}"
                    with open(tmp, "wb") as f:
                        f.write(data)
                    os.replace(tmp, path)
                except Exception:
                    pass
            return ret, data

        bass2jax.neuronx_cc_hook = cached_hook
        # If the plain hook was already installed on libneuronxla, refresh it.
        try:
            import libneuronxla

            if getattr(libneuronxla, "orig_neuronx_cc", None) is not None:
                libneuronxla.neuronx_cc = cached_hook
        except ImportError:
            pass
        _CACHE["cc_cached"] = True
    except Exception:
        pass


def kernel(x, fc_weights, gama):
    from concourse.bass_utils import run_bass_kernel_spmd

    _install_neff_cache()
    nc = _get_nc()
    in_maps = make_in_maps(x, fc_weights, gama)
    res = run_bass_kernel_spmd(nc, in_maps, core_ids=list(range(N_CORES)))
    out = np.concatenate([np.asarray(r["out"]) for r in res.results], axis=0)
    return np.ascontiguousarray(
        out.reshape(NFULL, C, 28, 28).astype(np.float32))


# revision 46
# speedup vs baseline: 1.0214x; 1.0214x over previous
"""Trainium2 Bass kernel for the topk-masking attention module.

Computation (per sample n):
    cams[k, hw] = relu(sum_c x[n, c, hw] * w[k, c])          # 1x1 conv, K=4
    thr[k]      = gama * max_hw(cams[k, :])
    dropped     = where(cams > thr, 0, cams)
    mean[hw]    = sum_k dropped[k, hw] / 4
    out[n,c,hw] = x[n,c,hw] * mean[hw]

Strategy (build_nc_v3, the shipped config): data-parallel over batch
N=32 across 8 NeuronCores (4 samples per core), with precision chosen
per data stream against the graded tolerance (rel_err < 2e-2):

- The host splits x = xh + xl/2048 with xh fp16 and xl the fp8-e4m3 of
  the scaled residual, and w into (wh fp16 | (w-wh)*2^11 fp16) plus
  w*2^5 in fp8.  HBM reads drop to 3 B/element (38.6 MB/core vs 51.4
  f32) while the conv stays effectively exact: the three single-pass
  matmul terms (wh*xh -> o_hi, wl*xh -> o_lo, w8*xl8 -> t3, each in its
  own base-0 PSUM tile, combined as o_hi + 2^-11 o_lo + 2^-16 t3 on
  ACT+DVE) reconstruct cams to ~7e-6 rms -- ZERO mask-threshold flips
  on the reference inputs, and 1-pass matmuls cost 1 PE cycle/row vs
  fp32's 4.
- The final multiply runs all-fp16 on DVE (2x mode), in place over the
  resident xh pair tiles, and the fp16 product is DMA'd out (25.7
  MB/core) and widened to f32 on the host after the gather.  Total HBM
  traffic 64.3 MB/core vs the f32 kernel's 102.8.
- DMA granularity: pair-sized loads/stores keep every HBM descriptor
  run at 12.5 KB/partition (8-piece loads measured 356 GB/s vs 401).
  Loads ride the sync ring, stores the scalar ring; two sample-NS-2
  pair stores are held back to bridge the final mask-chain stall.
- The mean matmul runs bf16 (masked/qlhs bf16), and the PSUM mean is
  staged to SBUF as fp16 by ACT so the multiply reads 16-bit operands.

Measured end to end: 281.1 us (f32 baseline) -> 196.5 us on 8-core
trn2, rel err 1.2e-3 (gate 2e-2), with DMA ~84% occupied at ~390 GB/s
per core and the PE ~70% busy underneath.
"""

import hashlib
import os
import sys

for _p in ("/opt/trn_rl_repo",):
    if _p not in sys.path:
        sys.path.insert(0, _p)

import numpy as np

N_CORES = 8
NFULL = 32            # full batch
NS = NFULL // N_CORES  # samples per core
C = 4096
K = 4
HW = 28 * 28          # 784
NCHUNK = C // 128     # 32
HALVES = ((0, 512), (512, HW))  # PSUM-bank-aligned column split

_CACHE = {}


def build_nc(n_pieces=8, x_bufs=16, out_bufs=8, cams_bufs=2, mean_bufs=2,
             store_engine="scalar", mean_to_sbuf=False, pe_filler=2,
             const_engine="scalar", hold_stores=2, spool_bufs=1,
             out_dt="float16", conv_fp32r=False, inplace_fp16=True,
             split_bf16=False, mean_dt="float32"):
    """Trace + schedule + compile the per-core Bass program.

    n_pieces: how many SBUF tiles one sample's x is split into (must
        divide 32); x_bufs slots of [128, 32/n_pieces, 784] each.
    out_dt: dtype of the DRAM output tensor.  float16 halves the store
        traffic (77.1 MB/core total vs 102.8 f32); the host widens the
        gathered result back to f32.  DVE writes the fp16 piece tiles
        (out_bufs slots) directly from the multiply.
    conv_fp32r: run the 1x1-conv matmuls with float32r operands
        (1 PE cycle/row vs fp32's 4 when the moving dim >= 256).
        Numerically a reduced-precision multiply -- flips some
        mask-threshold decisions; measure the final rel err on HW
        before trusting it.
    store_engine: which engine issues output DMAs ("sync"/"scalar"/
        "gpsimd") -- separate HWDGE ring from the loads avoids FIFO
        coupling.
    const_engine: ring for the w/gam/qlhs loads -- off the x-load ring
        so the first x piece transfer starts immediately.
    pe_filler: chunks of sample n+1 the PE runs ahead of sample n's mean
        matmul (in-order engine) to cover the relu/max/mask chain latency.
    hold_stores: defer this many stores of sample NS-2 and issue them at
        the start of sample NS-1's mask-chain window, bridging the DMA
        idle gap of that dependency stall.
    """
    from contextlib import ExitStack

    import concourse.bacc as bacc
    import concourse.tile as tile
    from concourse import mybir

    f32 = mybir.dt.float32
    bf16 = mybir.dt.bfloat16
    odt = getattr(mybir.dt, out_dt)
    # float32r is the same 4 bytes as float32; the BIR verifier requires the
    # whole producer chain (DRAM tensor -> DMA -> SBUF tile) to carry the
    # f32r tag for the matmult to consume it, so pick the dtype here.
    xdt = mybir.dt.float32r if conv_fp32r else f32
    nc = bacc.Bacc("TRN2", target_bir_lowering=False, debug=False,
                   num_devices=N_CORES)

    NP = n_pieces
    CPP = NCHUNK // NP  # chunks per piece

    if split_bf16:
        # x split on host into bf16 hi + lo planes (x == hi + lo to ~2^-17
        # relative): the conv runs as 3 single-pass bf16 matmuls per chunk
        # (wh*xh + wl*xh + wh*xl, dropping the ~2^-16-relative wl*xl term)
        # instead of fp32's effective 4 cycles/row -- same HBM read bytes,
        # 25% less PE time, and zero mask-threshold flips (measured: cams
        # rms error 4e-6 vs a flip band of ~1e-3).  The final multiply uses
        # only x_hi (out = fp16(x_hi * mean), ~2e-3 per-element) and runs
        # in place over the x_hi tile.
        xh_d = nc.dram_tensor("xh", [NS, C, HW], bf16, kind="ExternalInput")
        xl_d = nc.dram_tensor("xl", [NS, C, HW], bf16, kind="ExternalInput")
        w_d = nc.dram_tensor("w", [128, NCHUNK, 2, K], bf16,
                             kind="ExternalInput")
    else:
        x_d = nc.dram_tensor("x", [NS, C, HW], xdt, kind="ExternalInput")
        w_d = nc.dram_tensor("w", [128, NCHUNK, K], xdt, kind="ExternalInput")
    gam_d = nc.dram_tensor("gam", [K, 1], f32, kind="ExternalInput")
    qlhs_d = nc.dram_tensor("qlhs", [K, 128], f32, kind="ExternalInput")
    out_d = nc.dram_tensor("out", [NS, C, HW], odt, kind="ExternalOutput")

    # [NS, C, HW] viewed as [NS, 128(part), NCHUNK, HW]: partition p holds
    # the NCHUNK *adjacent* channels c = p*NCHUNK + j.  Each (partition,
    # piece) DMA run is then CPP*HW*dtype contiguous bytes and the w
    # host packing in make_in_maps is a plain reshape with the same mapping.
    if split_bf16:
        xh_v = xh_d.ap().rearrange("n (p j) hw -> n p j hw", p=128, j=NCHUNK)
        xl_v = xl_d.ap().rearrange("n (p j) hw -> n p j hw", p=128, j=NCHUNK)
    else:
        x_v = x_d.ap().rearrange("n (p j) hw -> n p j hw", p=128, j=NCHUNK)
    out_v = out_d.ap().rearrange("n (p j) hw -> n p j hw", p=128, j=NCHUNK)

    if isinstance(store_engine, (list, tuple)):
        store_engs = [getattr(nc, e) for e in store_engine]
    else:
        store_engs = [getattr(nc, store_engine)]
    const_eng = getattr(nc, const_engine)

    with tile.TileContext(nc) as tc, ExitStack() as ctx:
        consts = ctx.enter_context(tc.tile_pool(name="consts", bufs=1))
        # split mode allocates pair-sized load tiles, so halve the buf count
        # to keep the same two-samples-resident SBUF footprint.
        xb = x_bufs // 2 if split_bf16 else x_bufs
        xpool = ctx.enter_context(tc.tile_pool(name="xpool", bufs=xb))
        if split_bf16:
            lpool = ctx.enter_context(tc.tile_pool(name="lpool", bufs=xb))
        opool = ctx.enter_context(tc.tile_pool(name="opool", bufs=out_bufs))
        spool = ctx.enter_context(tc.tile_pool(name="spool", bufs=spool_bufs))
        tpool = ctx.enter_context(tc.tile_pool(name="tpool", bufs=2))
        cpsum = ctx.enter_context(
            tc.tile_pool(name="cpsum", bufs=cams_bufs, space="PSUM"))
        mpsum = ctx.enter_context(
            tc.tile_pool(name="mpsum", bufs=mean_bufs, space="PSUM"))

        xq_all = {}
        xl_all = {}
        ot_all = {}
        cams_all = {}

        def emit_loads(n):
            xq_all[n] = []
            xl_all[n] = []
            if split_bf16:
                # Load hi/lo in piece PAIRS: [128, 2*CPP, HW] bf16 tiles give
                # the same 12.5KB-per-partition contiguous DMA runs as the
                # f32 path (8 piece-sized loads measured 356 GB/s vs 401 for
                # pair-sized).  Multiply/store keep piece granularity via
                # half-tile views.
                for q2 in range(NP // 2):
                    sl = slice(q2 * 2 * CPP, (q2 + 1) * 2 * CPP)
                    t = xpool.tile([128, 2 * CPP, HW], bf16, tag="xh",
                                   name=f"xh_{n}_{q2}")
                    nc.sync.dma_start(t[:], xh_v[n][:, sl, :])
                    tl = lpool.tile([128, 2 * CPP, HW], bf16, tag="xl",
                                    name=f"xl_{n}_{q2}")
                    nc.sync.dma_start(tl[:], xl_v[n][:, sl, :])
                    for h in range(2):
                        hs = slice(h * CPP, (h + 1) * CPP)
                        xq_all[n].append(t[:, hs, :])
                        xl_all[n].append(tl[:, hs, :])
                return
            for q in range(NP):
                sl = slice(q * CPP, (q + 1) * CPP)
                t = xpool.tile([128, CPP, HW], xdt, tag="xq",
                               name=f"xq_{n}_{q}")
                nc.sync.dma_start(t[:], x_v[n][:, sl, :])
                xq_all[n].append(t)

        # x loads first in trace order; consts ride a separate ring.
        emit_loads(0)
        if split_bf16:
            w_sb = consts.tile([128, NCHUNK, 2, K], bf16, name="w_sb")
        else:
            w_sb = consts.tile([128, NCHUNK, K], xdt, name="w_sb")
        const_eng.dma_start(w_sb[:], w_d.ap())
        gam_sb = consts.tile([K, 1], f32, name="gam_sb")
        const_eng.dma_start(gam_sb[:], gam_d.ap())
        qlhs_sb = consts.tile([K, 128], f32, name="qlhs_sb")
        const_eng.dma_start(qlhs_sb[:], qlhs_d.ap())

        def as_f32(ap):
            # DVE consumes the f32r-tagged tiles as plain f32 bits.
            return ap.bitcast(f32) if conv_fp32r else ap

        def fp16_view(t):
            if split_bf16:
                # bf16 -> fp16 is the same element size: a plain in-place
                # dtype alias over the x_hi (half-tile view) AP.
                return t.bitcast(odt)
            # fp16 alias over the FIRST HALF of an f32 tile's bytes: the DVE
            # multiply streams in element order, so writing element i at byte
            # 2i while reading it at byte 4i never overtakes the reads.
            flat = t[:].rearrange("p j hw -> p (j hw)")
            return flat.bitcast(odt)[:, :CPP * HW].rearrange(
                "p (j hw) -> p j hw", j=CPP, hw=HW)

        def emit_chunk_mms(n, j_lo, j_hi):
            cams = cams_all[n]
            xq = xq_all[n]
            for j in range(j_lo, j_hi):
                q, jj = divmod(j, CPP)
                if split_bf16:
                    # wh*xh + wl*xh + wh*xl, accumulated in f32 PSUM.  The
                    # wh stationary is loaded twice back to back so the
                    # ldweights of each matmul hides under the previous
                    # matmul's rows (double-buffered PE weights).
                    terms = ((0, xq_all[n][q]), (1, xq_all[n][q]),
                             (0, xl_all[n][q]))
                    for t, (wi, src) in enumerate(terms):
                        for c0, c1 in HALVES:
                            nc.tensor.matmul(
                                cams[:, c0:c1],
                                w_sb[:, j, wi, :],
                                src[:, jj, c0:c1],
                                start=(j == 0 and t == 0),
                                stop=(j == NCHUNK - 1 and t == 2),
                            )
                else:
                    for c0, c1 in HALVES:
                        nc.tensor.matmul(
                            cams[:, c0:c1],
                            w_sb[:, j, :],
                            xq[q][:, jj, c0:c1],
                            start=(j == 0),
                            stop=(j == NCHUNK - 1),
                        )

        def emit_store(n, q):
            store_engs[q % len(store_engs)].dma_start(
                out_v[n][:, q * CPP:(q + 1) * CPP, :], ot_all[n][q])

        for n in range(NS):
            if n not in cams_all:
                cams_all[n] = cpsum.tile([K, HW], f32, tag="cams",
                                         name=f"cams_{n}")
            emit_chunk_mms(n, pe_filler if n > 0 else 0, NCHUNK)
            cams = cams_all[n]
            xq = xq_all[n]

            # Next sample's loads ahead of the mask chain in trace order so
            # the load ring's issue queue never drains behind it.
            if n + 1 < NS:
                emit_loads(n + 1)

            # relu on ACT (PSUM -> SBUF)
            r = spool.tile([K, HW], f32, tag="r", name=f"r_{n}")
            nc.scalar.activation(r[:], cams[:],
                                 mybir.ActivationFunctionType.Relu)
            # Held-back stores of sample n-1: dependency-free by now, they
            # keep the DMA engines fed through this sample's mask-chain
            # stall (the final sample especially).
            if n == NS - 1:
                for q in range(NP - hold_stores, NP):
                    emit_store(n - 1, q)
            # per-channel spatial max
            rmax = tpool.tile([K, 1], f32, tag="rmax", name=f"rmax_{n}")
            nc.vector.tensor_reduce(rmax[:], r[:], axis=mybir.AxisListType.X,
                                    op=mybir.AluOpType.max)
            # thr = gama * max
            thr = tpool.tile([K, 1], f32, tag="thr", name=f"thr_{n}")
            nc.vector.tensor_scalar(thr[:], rmax[:], gam_sb[:], None,
                                    op0=mybir.AluOpType.mult)
            # masked = (r <= thr) * r
            masked = spool.tile([K, HW], f32, tag="masked", name=f"masked_{n}")
            nc.vector.scalar_tensor_tensor(masked[:], r[:], thr[:], r[:],
                                           op0=mybir.AluOpType.is_le,
                                           op1=mybir.AluOpType.mult)
            # Keep PE busy while the DVE mask for sample n completes:
            # emit the first pe_filler chunk matmuls of sample n+1 ahead of
            # sample n's mean matmul in PE program order (in-order engine,
            # head-of-line blocking otherwise; also avoids a HAM idle gap).
            if n + 1 < NS and pe_filler:
                cams_all[n + 1] = cpsum.tile([K, HW], f32, tag="cams",
                                             name=f"cams_{n + 1}")
                emit_chunk_mms(n + 1, 0, pe_filler)

            # mean over k, broadcast to 128 partitions: qlhs (0.25) matmul
            meanb = mpsum.tile([128, HW], f32, tag="meanb", name=f"meanb_{n}")
            for c0, c1 in HALVES:
                nc.tensor.matmul(meanb[:, c0:c1], qlhs_sb[:],
                                 masked[:, c0:c1], start=True, stop=True)

            mean_src = meanb
            if mean_to_sbuf:
                # PSUM -> SBUF on ACT: GpSimd cannot read PSUM, and SBUF
                # operands are cheaper for DVE too.  mean_dt=float16 halves
                # the DVE read bytes of the broadcast operand (the mean is
                # re-rounded anyway by the fp16 store).
                mdt = getattr(mybir.dt, mean_dt)
                mean_sb = spool.tile([128, HW], mdt, tag="mean_sb",
                                     name=f"mean_sb_{n}")
                nc.scalar.activation(mean_sb[:], meanb[:],
                                     mybir.ActivationFunctionType.Copy)
                mean_src = mean_sb

            mb = mean_src.unsqueeze(1).broadcast_to([128, CPP, HW])
            ot_all[n] = []
            for q in range(NP):
                if inplace_fp16:
                    ot = fp16_view(xq[q])
                else:
                    ot = opool.tile([128, CPP, HW], odt, tag="ot",
                                    name=f"ot_{n}_{q}")[:]
                ot_all[n].append(ot)
                src = xq[q] if split_bf16 else as_f32(xq[q][:])
                nc.vector.tensor_tensor(ot, src, mb,
                                        op=mybir.AluOpType.mult)
                if n == NS - 2 and q >= NP - hold_stores:
                    continue  # deferred: see the hold_stores block above
                emit_store(n, q)

    nc.compile()
    nc._kernel_cfg = {"split_bf16": split_bf16}
    return nc


def build_nc_v3(cams_bufs=1, mean_bufs=1,
                store_engine="scalar", pe_filler=0, const_engine="scalar",
                hold_stores=2, spool_bufs=1, x_bufs=8, l_bufs=4,
                mean_dt="float16", mean_mm_bf16=False, t3_batch=False,
                pair_out=False, head_loads=False):
    """v3: fp16-hi + scaled-fp8-lo x planes, double-wide stationary.

    Host splits x = xh + xl/2048 (xh fp16, xl float8e4m3 of the scaled
    residual) and w into (wh fp16 | (w-wh)*2048 fp16) plus w*32 in fp8.
    Per chunk the conv is TWO single-pass matmuls instead of fp32's
    2x2-pass pair:
      rows 0:8  += [wh | wl*2^11]^T xh   (fp16, one [128,8] stationary --
                                          both w terms share the rows)
      rows 8:12 += (w*2^5)^T (xl*2^11)   (fp8)
    cams = o[0:4] + 2^-11 o[4:8] + 2^-16 o[8:12]  (two DVE ops)
    Measured on host against the reference inputs: zero mask-threshold
    flips, final rel err 3.6e-4.  Reads drop to 3 B/elem (38.6 MB/core),
    writes 25.7 MB fp16: 64.3 MB total HBM vs the f32 kernel's 102.8.
    The final multiply is all-fp16 (2x DVE mode), in place over the xh
    tiles, and the store DMAs run at piece granularity behind it.
    """
    from contextlib import ExitStack

    import concourse.bacc as bacc
    import concourse.tile as tile
    from concourse import mybir

    f32 = mybir.dt.float32
    f16 = mybir.dt.float16
    bf16 = mybir.dt.bfloat16
    f8 = mybir.dt.float8e4
    nc = bacc.Bacc("TRN2", target_bir_lowering=False, debug=False,
                   num_devices=N_CORES)

    NP = 8
    CPP = NCHUNK // NP  # 4 chunks per logical piece
    mmdt = bf16 if mean_mm_bf16 else f32

    xh_d = nc.dram_tensor("xh", [NS, C, HW], f16, kind="ExternalInput")
    xl_d = nc.dram_tensor("xl", [NS, C, HW], f8, kind="ExternalInput")
    w12_d = nc.dram_tensor("w12", [128, NCHUNK, 2 * K], f16,
                           kind="ExternalInput")
    w3_d = nc.dram_tensor("w3", [128, NCHUNK, K], f8, kind="ExternalInput")
    gam_d = nc.dram_tensor("gam", [K, 1], f32, kind="ExternalInput")
    qlhs_d = nc.dram_tensor("qlhs", [K, 128], mmdt, kind="ExternalInput")
    out_d = nc.dram_tensor("out", [NS, C, HW], f16, kind="ExternalOutput")

    xh_v = xh_d.ap().rearrange("n (p j) hw -> n p j hw", p=128, j=NCHUNK)
    xl_v = xl_d.ap().rearrange("n (p j) hw -> n p j hw", p=128, j=NCHUNK)
    out_v = out_d.ap().rearrange("n (p j) hw -> n p j hw", p=128, j=NCHUNK)

    if isinstance(store_engine, (list, tuple)):
        store_engs = [getattr(nc, e) for e in store_engine]
    else:
        store_engs = [getattr(nc, store_engine)]
    const_eng = getattr(nc, const_engine)

    with tile.TileContext(nc) as tc, ExitStack() as ctx:
        consts = ctx.enter_context(tc.tile_pool(name="consts", bufs=1))
        xpool = ctx.enter_context(tc.tile_pool(name="xpool", bufs=x_bufs))
        lpool = ctx.enter_context(tc.tile_pool(name="lpool", bufs=l_bufs))
        spool = ctx.enter_context(tc.tile_pool(name="spool", bufs=spool_bufs))
        tpool = ctx.enter_context(tc.tile_pool(name="tpool", bufs=2))
        cpsum = ctx.enter_context(
            tc.tile_pool(name="cpsum", bufs=cams_bufs, space="PSUM"))
        mpsum = ctx.enter_context(
            tc.tile_pool(name="mpsum", bufs=mean_bufs, space="PSUM"))

        xq_all = {}   # per sample: 8 half-views [128, CPP, HW] f16 of pairs
        xp_all = {}   # per sample: 4 full pair APs [128, 2*CPP, HW] f16
        xl_all = {}   # per sample: 8 quarter-views [128, CPP, HW] f8 of quads
        cams_all = {}
        t3_all = {}

        def emit_pair(n, q2, eng=None):
            sl = slice(q2 * 2 * CPP, (q2 + 1) * 2 * CPP)
            t = xpool.tile([128, 2 * CPP, HW], f16, tag="xh",
                           name=f"xh_{n}_{q2}")
            (eng or nc.sync).dma_start(t[:], xh_v[n][:, sl, :])
            xp_all[n].append(t[:])
            for h in range(2):
                xq_all[n].append(t[:, h * CPP:(h + 1) * CPP, :])

        def emit_quad(n, q4, eng=None):
            sl = slice(q4 * 4 * CPP, (q4 + 1) * 4 * CPP)
            tl = lpool.tile([128, 4 * CPP, HW], f8, tag="xl",
                            name=f"xl_{n}_{q4}")
            (eng or nc.sync).dma_start(tl[:], xl_v[n][:, sl, :])
            for h in range(4):
                xl_all[n].append(tl[:, h * CPP:(h + 1) * CPP, :])

        def emit_loads(n, head_eng=None):
            xq_all[n] = []
            xp_all[n] = []
            xl_all[n] = []
            # fp16 hi pairs + fp8 lo quads, all 12.5 KB/partition runs,
            # ordered by first PE use: the conv's fp8 term of piece 0 needs
            # quad 0 right after pair 0 (loading all pairs first measured
            # the first matmul gated at ~16us instead of ~5).  For sample 0
            # the first pair+quad can ride the const ring so the PE's first
            # wait clears as soon as they land, decoupled from the sync
            # queue's longer backlog.
            emit_pair(n, 0, head_eng)
            emit_quad(n, 0, head_eng)
            emit_pair(n, 1)
            emit_pair(n, 2)
            emit_quad(n, 1)
            emit_pair(n, 3)

        # consts first so they head the scalar queue, then sample 0's first
        # pair+quad on the same ring (see emit_loads).
        w12_sb = consts.tile([128, NCHUNK, 2 * K], f16, name="w12_sb")
        const_eng.dma_start(w12_sb[:], w12_d.ap())
        w3_sb = consts.tile([128, NCHUNK, K], f8, name="w3_sb")
        const_eng.dma_start(w3_sb[:], w3_d.ap())
        gam_sb = consts.tile([K, 1], f32, name="gam_sb")
        const_eng.dma_start(gam_sb[:], gam_d.ap())
        qlhs_sb = consts.tile([K, 128], mmdt, name="qlhs_sb")
        const_eng.dma_start(qlhs_sb[:], qlhs_d.ap())
        emit_loads(0, head_eng=(const_eng if head_loads else None))

        def emit_chunk_mms(n, j_lo, j_hi):
            o_hi, o_lo = cams_all[n]
            t3 = t3_all[n]
            for j in range(j_lo, j_hi):
                q, jj = divmod(j, CPP)
                for c0, c1 in HALVES:
                    # DVE PSUM reads need partition base 0 mod 32, so the
                    # wh and wl*2^11 terms go to separate base-0 tiles.
                    for out_t, w_lo_col in ((o_hi, 0), (o_lo, K)):
                        nc.tensor.matmul(
                            out_t[:, c0:c1],
                            w12_sb[:, j, w_lo_col:w_lo_col + K],
                            xq_all[n][q][:, jj, c0:c1],
                            start=(j == 0),
                            stop=(j == NCHUNK - 1),
                        )
                    if not t3_batch:
                        nc.tensor.matmul(
                            t3[:, c0:c1],
                            w3_sb[:, j, :],
                            xl_all[n][q][:, jj, c0:c1],
                            start=(j == 0),
                            stop=(j == NCHUNK - 1),
                        )
                if t3_batch and (j + 1) % CPP == 0:
                    # batch the fp8 term per piece to cut stationary-dtype
                    # flip-flopping on the PE
                    for j2 in range(j - CPP + 1, j + 1):
                        q2, jj2 = divmod(j2, CPP)
                        for c0, c1 in HALVES:
                            nc.tensor.matmul(
                                t3[:, c0:c1],
                                w3_sb[:, j2, :],
                                xl_all[n][q2][:, jj2, c0:c1],
                                start=(j2 == 0),
                                stop=(j2 == NCHUNK - 1),
                            )

        def emit_store(n, q, fine=False):
            if pair_out and not fine:
                sl = slice(q * 2 * CPP, (q + 1) * 2 * CPP)
                store_engs[q % len(store_engs)].dma_start(
                    out_v[n][:, sl, :], xp_all[n][q])
                return
            store_engs[q % len(store_engs)].dma_start(
                out_v[n][:, q * CPP:(q + 1) * CPP, :], xq_all[n][q])

        NOUT = NP // 2 if pair_out else NP  # multiply/store granules

        def alloc_psum(n):
            cams_all[n] = (
                cpsum.tile([K, HW], f32, tag="ohi", name=f"ohi_{n}"),
                cpsum.tile([K, HW], f32, tag="olo", name=f"olo_{n}"),
            )
            t3_all[n] = cpsum.tile([K, HW], f32, tag="t3", name=f"t3_{n}")

        for n in range(NS):
            if n not in cams_all:
                alloc_psum(n)
            emit_chunk_mms(n, pe_filler if n > 0 else 0, NCHUNK)
            o_hi, o_lo = cams_all[n]

            if n + 1 < NS:
                emit_loads(n + 1)

            # cams = o_hi + 2^-11 o_lo + 2^-16 o_t3.  DVE may read only ONE
            # PSUM operand per op, so ACT stages the scaled o_lo to SBUF.
            tmp = spool.tile([K, HW], f32, tag="ctmp", name=f"ctmp_{n}")
            nc.scalar.activation(tmp[:], o_lo[:],
                                 mybir.ActivationFunctionType.Copy,
                                 scale=1.0 / 2048.0)
            tmp2 = spool.tile([K, HW], f32, tag="ctmp2", name=f"ctmp2_{n}")
            nc.vector.scalar_tensor_tensor(
                tmp2[:], o_hi[:], 1.0, tmp[:],
                op0=mybir.AluOpType.mult, op1=mybir.AluOpType.add)
            csb = spool.tile([K, HW], f32, tag="csb", name=f"csb_{n}")
            nc.vector.scalar_tensor_tensor(
                csb[:], t3_all[n][:], 1.0 / 65536.0, tmp2[:],
                op0=mybir.AluOpType.mult, op1=mybir.AluOpType.add)
            # relu on ACT (SBUF -> SBUF)
            r = spool.tile([K, HW], f32, tag="r", name=f"r_{n}")
            nc.scalar.activation(r[:], csb[:],
                                 mybir.ActivationFunctionType.Relu)
            if n == NS - 1:
                for q in range(NOUT - hold_stores, NOUT):
                    emit_store(n - 1, q)
            rmax = tpool.tile([K, 1], f32, tag="rmax", name=f"rmax_{n}")
            nc.vector.tensor_reduce(rmax[:], r[:], axis=mybir.AxisListType.X,
                                    op=mybir.AluOpType.max)
            thr = tpool.tile([K, 1], f32, tag="thr", name=f"thr_{n}")
            nc.vector.tensor_scalar(thr[:], rmax[:], gam_sb[:], None,
                                    op0=mybir.AluOpType.mult)
            masked = spool.tile([K, HW], mmdt, tag="masked",
                                name=f"masked_{n}")
            nc.vector.scalar_tensor_tensor(masked[:], r[:], thr[:], r[:],
                                           op0=mybir.AluOpType.is_le,
                                           op1=mybir.AluOpType.mult)
            if n + 1 < NS and pe_filler:
                alloc_psum(n + 1)
                emit_chunk_mms(n + 1, 0, pe_filler)

            meanb = mpsum.tile([128, HW], f32, tag="meanb", name=f"meanb_{n}")
            for c0, c1 in HALVES:
                nc.tensor.matmul(meanb[:, c0:c1], qlhs_sb[:],
                                 masked[:, c0:c1], start=True, stop=True)

            # PSUM -> SBUF fp16 mean so the multiply runs all-16-bit on DVE
            mdt = getattr(mybir.dt, mean_dt)
            mean_sb = spool.tile([128, HW], mdt, tag="mean_sb",
                                 name=f"mean_sb_{n}")
            nc.scalar.activation(mean_sb[:], meanb[:],
                                 mybir.ActivationFunctionType.Copy)

            # Last sample: piece granularity.  Its multiplies+stores are the
            # exposed tail (no later compute to hide behind); halving the
            # granule releases store work to the otherwise-idle DMA at twice
            # the rate.
            fine = pair_out and n == NS - 1
            gran = 2 * CPP if (pair_out and not fine) else CPP
            nout = NP // 2 if (pair_out and not fine) else NP
            mb = mean_sb.unsqueeze(1).broadcast_to([128, gran, HW])
            srcs = xq_all[n] if (fine or not pair_out) else xp_all[n]
            for q in range(nout):
                # in place: fp16 xh view * fp16 mean -> same fp16 bytes
                nc.vector.tensor_tensor(srcs[q], srcs[q], mb,
                                        op=mybir.AluOpType.mult)
                if n == NS - 2 and q >= nout - hold_stores:
                    continue
                emit_store(n, q, fine=fine)

    nc.compile()
    nc._kernel_cfg = {"v3": True, "mean_mm_bf16": mean_mm_bf16}
    return nc


def _get_nc():
    if "nc" not in _CACHE:
        _CACHE["nc"] = build_nc_v3(t3_batch=True, hold_stores=2,
                                   mean_mm_bf16=True, pair_out=True)
    return _CACHE["nc"]


def make_in_maps(x, fc_weights, gama):
    """Shard/pack full numpy inputs into per-core input maps."""
    cfg = getattr(_get_nc(), "_kernel_cfg", {})
    x = np.ascontiguousarray(np.asarray(x, dtype=np.float32).reshape(NFULL, C, HW))
    fcw = np.asarray(fc_weights, dtype=np.float32).reshape(K, C)
    gam4 = np.full((K, 1), np.float32(np.asarray(gama)), dtype=np.float32)
    qlhs = np.full((K, 128), 0.25, dtype=np.float32)
    in_maps = []
    if cfg.get("v3"):
        import ml_dtypes

        e4 = ml_dtypes.float8_e4m3fn
        xh = x.astype(np.float16)
        xl8 = ((x - xh.astype(np.float32)) * 2048.0).astype(e4)
        wh = fcw.astype(np.float16)
        wls = ((fcw - wh.astype(np.float32)) * 2048.0).astype(np.float16)
        # w12[p, j, 0:4] = wh[k, c], w12[p, j, 4:8] = (w-wh)*2^11;
        # w3[p, j, k] = w[k, c]*2^5 in fp8; channel c = p*NCHUNK + j
        w12 = np.ascontiguousarray(np.concatenate(
            [wh.T.reshape(128, NCHUNK, K),
             wls.T.reshape(128, NCHUNK, K)], axis=2))
        w3 = np.ascontiguousarray(
            (fcw.T.reshape(128, NCHUNK, K) * 32.0).astype(e4))
        if cfg.get("mean_mm_bf16"):
            qlhs = qlhs.astype(ml_dtypes.bfloat16)
        for c in range(N_CORES):
            in_maps.append({
                "xh": xh[c * NS:(c + 1) * NS],
                "xl": xl8[c * NS:(c + 1) * NS],
                "w12": w12,
                "w3": w3,
                "gam": gam4,
                "qlhs": qlhs,
            })
        return in_maps
    if cfg.get("split_bf16"):
        import ml_dtypes

        bf16 = ml_dtypes.bfloat16
        xh = x.astype(bf16)
        xl = (x - xh.astype(np.float32)).astype(bf16)
        wh = fcw.astype(bf16)
        wl = (fcw - wh.astype(np.float32)).astype(bf16)
        # w2[p, j, i, k] = (wh if i == 0 else wl)[k, p*NCHUNK + j]
        w2 = np.ascontiguousarray(
            np.stack([wh.T.reshape(128, NCHUNK, K),
                      wl.T.reshape(128, NCHUNK, K)], axis=2))
        for c in range(N_CORES):
            in_maps.append({
                "xh": xh[c * NS:(c + 1) * NS],
                "xl": xl[c * NS:(c + 1) * NS],
                "w": w2,
                "gam": gam4,
                "qlhs": qlhs,
            })
        return in_maps
    # w_arr[p, j, k] = fcw[k, p*NCHUNK + j]  (channel c = p*NCHUNK + j,
    # matching the x view in build_nc)
    w_arr = np.ascontiguousarray(fcw.T.reshape(128, NCHUNK, K))
    for c in range(N_CORES):
        in_maps.append({
            "x": x[c * NS:(c + 1) * NS],
            "w": w_arr,
            "gam": gam4,
            "qlhs": qlhs,
        })
    return in_maps


def _strip_debug(obj):
    """Recursively blank debug-only fields (file paths / tracebacks) so the
    cache key is independent of where kernel.py lives on disk."""
    if isinstance(obj, dict):
        return {
            k: ("" if k in ("filename", "ant_traceback") else _strip_debug(v))
            for k, v in obj.items()
        }
    if isinstance(obj, list):
        return [_strip_debug(v) for v in obj]
    return obj


def _bass_module_cache_key(code, code_format):
    """Semantic cache key for a bass_exec HLO module, or None.

    Hashes the embedded BIR with debug-only fields blanked, plus the
    IO-name/arch config.  Any semantic difference changes the key; a
    path-only difference (same kernel traced from another directory)
    does not.
    """
    import base64
    import json

    if b"bass_exec" not in bytes(code) or bytes(code_format) != b"hlo":
        return None
    import libneuronxla.proto.hlo_pb2 as hlo_pb2
    from concourse import bass2jax

    proto = hlo_pb2.HloModuleProto.FromString(bytes(code))
    cfgs = [
        ins.backend_config
        for comp in proto.computations
        for ins in comp.instructions
        if ins.opcode == "custom-call" and ins.custom_call_target == "bass_exec"
    ]
    if len(cfgs) != 1:
        return None
    config = json.loads(base64.standard_b64decode(cfgs[0]))
    decomp = getattr(bass2jax, "_decompress_ant_bir", None)
    if decomp is None:
        return None
    bir = json.loads(decomp(config["ant_bir"]))
    h = hashlib.sha256()
    h.update(json.dumps(_strip_debug(bir), sort_keys=True).encode())
    h.update(json.dumps(
        [config.get("in_names"), config.get("out_names"),
         config.get("arch"), proto.name],
        sort_keys=True).encode())
    return h.hexdigest()


def _install_neff_cache():
    """Wrap concourse's neuronx_cc hook with a content-keyed NEFF cache.

    The stock hook recompiles the NEFF from scratch in every process
    (minutes for this kernel); the emitted BIR is deterministic modulo
    debug file paths, so a debug-stripped content hash makes repeat
    compiles of the identical module instant.
    """
    if _CACHE.get("cc_cached"):
        return
    try:
        from concourse import bass2jax

        inner = bass2jax.neuronx_cc_hook
        cache_dir = os.path.expanduser("~/.cache/bass_neff_cache")
        os.makedirs(cache_dir, exist_ok=True)

        def cached_hook(code, code_format, platform_version, file_prefix):
            path = None
            try:
                key = _bass_module_cache_key(code, code_format)
                if key is not None:
                    path = os.path.join(cache_dir, key)
                    if os.path.exists(path):
                        with open(path, "rb") as f:
                            return 0, f.read()
            except Exception:
                path = None
            ret, data = inner(code, code_format, platform_version, file_prefix)
            if path is not None and ret == 0:
                try:
                    tmp = f"{path}.tmp{os.getpid()}"
                    with open(tmp, "wb") as f:
                        f.write(data)
                    os.replace(tmp, path)
                except Exception:
                    pass
            return ret, data

        bass2jax.neuronx_cc_hook = cached_hook
        # If the plain hook was already installed on libneuronxla, refresh it.
        try:
            import libneuronxla

            if getattr(libneuronxla, "orig_neuronx_cc", None) is not None:
                libneuronxla.neuronx_cc = cached_hook
        except ImportError:
            pass
        _CACHE["cc_cached"] = True
    except Exception:
        pass


def kernel(x, fc_weights, gama):
    from concourse.bass_utils import run_bass_kernel_spmd

    _install_neff_cache()
    nc = _get_nc()
    in_maps = make_in_maps(x, fc_weights, gama)
    res = run_bass_kernel_spmd(nc, in_maps, core_ids=list(range(N_CORES)))
    out = np.concatenate([np.asarray(r["out"]) for r in res.results], axis=0)
    return np.ascontiguousarray(
        out.reshape(NFULL, C, 28, 28).astype(np.float32))
